# revision 19
# baseline (speedup 1.0000x reference)
"""DisagreementRegularizer Trainium2 kernel.

reference math:
    xn = x / max(||x||_2 along d, eps)
    sim[b,q,p] = xn[b,q,:] . xn[b,p,:]
    out[b] = -mean_{q,p} sim  =  -(1/Q^2) * || sum_q xn[b,q,:] ||^2

Per batch b (on device):
    sumsq[q] = sum_d x[q,d]^2            (ACT Square big-call, DVE segmented reduce)
    rnorm[q] = sqrt(1/sumsq[q])          (DVE reciprocal + ACT Sqrt -> fp16)
    s[d]     = sum_q rnorm[q]*x[q,d]     (PE matmul, rnorm as stationary weights)
Host: out[b] = -(1/Q^2) * sum_d s[b,d]^2   (tiny: 16x256 per core)

All activations used (square, sqrt, copy) live in the single table set
sqrt_and_others; a dummy Sqrt is emitted first so that set is chosen at
the first activation -> exactly one ACT_TABLE_LOAD. The rnorm chain runs
under tc.high_priority() so the tiny ops that unblock the matmuls are
not scheduled behind later groups' big squares. Each group gets a
2KB-bank-aligned PSUM region (Tile's bank tracker is partition-blind,
so bank sharing between groups creates false serialization).

x is cast fp32->fp16 during the DMA load (SWDGE cast) so the matmuls run
single-pass at 1 cycle/row instead of fp32's two half-rate passes.

Sharding: pure data parallel, batch dim 128 -> 16 per core across 8 cores.
"""

import numpy as np

B, Q, D = 128, 512, 256
N_CORES = 8
BL = B // N_CORES  # 16 batches per core
CHUNKS = 4  # Q = 512 = 128 partitions x 4 chunks
# 2-batch groups pace the pipeline finely; 1-batch tail groups shorten the
# serial chain after the last DMA lands
GROUP_SIZES = [2, 2, 2, 2, 2, 2, 1, 1, 1, 1]
# 512-col (2KB PSUM bank) aligned column base for each group's s vectors;
# only g7/g8 share a bank (adjacent in time anyway)
PS_BASES = [0, 512, 1024, 1536, 2048, 2560, 3072, 3328, 3584, 3840]
EPS = 1e-12


def _build(nc):
    import concourse.mybir as mybir
    import concourse.tile as tile

    f32 = mybir.dt.float32
    f16 = mybir.dt.float16

    x_d = nc.dram_tensor("x", [BL, Q, D], f32, kind="ExternalInput").ap()
    s_d = nc.dram_tensor("s_out", [BL, D], f32, kind="ExternalOutput").ap()

    with tile.TileContext(nc) as tc:
        with (
            tc.tile_pool(name="xp", bufs=1) as xp,
            tc.tile_pool(name="sqp", bufs=6) as sqp,
            tc.tile_pool(name="small", bufs=2 * len(GROUP_SIZES)) as small,
            tc.tile_pool(name="fin", bufs=3) as fin,
            tc.tile_pool(name="ps", bufs=1, space="PSUM") as psp,
        ):
            s_ps = psp.tile([1, BL * D], f32)  # 16 * 256 = 4096 fp32 = all of PSUM

            # issue every group's load upfront so the DMA stream is dense
            # from the first possible cycle; cast fp32->fp16 in the DMA.
            # partition p holds rows 4p..4p+3 of each batch.
            x_tiles = []
            b0 = 0
            for g, GB in enumerate(GROUP_SIZES):
                x_t = xp.tile([128, GB, CHUNKS, D], f16, tag=f"x_t{g}")
                src = x_d[b0 : b0 + GB].rearrange("b (p c) d -> p b c d", p=128)
                nc.gpsimd.dma_start(out=x_t[:], in_=src)
                x_tiles.append(x_t)
                b0 += GB

            # dummy Sqrt: pins the activation table set to sqrt_and_others
            # (which also contains square and copy)
            dummy = small.tile([1, 1], f32, tag="dummy")
            nc.vector.memset(dummy[:], 1.0)
            dummy2 = small.tile([1, 1], f32, tag="dummy2")
            nc.scalar.activation(
                out=dummy2[:], in_=dummy[:], func=mybir.ActivationFunctionType.Sqrt
            )

            b0 = 0
            prev_recip = None
            sqrt_hist = []
            for g, GB in enumerate(GROUP_SIZES):
                x_t = x_tiles[g]
                # square the whole group tile in one ACT call
                sq = sqp.tile([128, GB * CHUNKS * D], f16, tag="sq")
                sq_i = nc.scalar.activation(
                    out=sq[:],
                    in_=x_t[:].rearrange("p b c d -> p (b c d)"),
                    func=mybir.ActivationFunctionType.Square,
                )
                if len(sqrt_hist) >= 2:
                    # keep ACT interleaved with 2 groups of slack: the tiny
                    # sqrt of group g-2 must not get scheduled behind this
                    # (and every later) square, but forcing g-1's would
                    # serialize ACT against the DVE reduce chain
                    tile.add_dep_helper(
                        sq_i.ins, sqrt_hist[-2].ins, sync=False,
                        reason="interleave rnorm sqrt between squares",
                    )
                # segmented reduce -> sumsq per row (GB*4 segments of 256)
                sumsq = small.tile([128, GB * CHUNKS], f32, tag="sumsq")
                red_i = nc.vector.tensor_reduce(
                    out=sumsq[:],
                    in_=sq[:].rearrange("p (s d) -> p s d", d=D),
                    axis=mybir.AxisListType.X,
                    op=mybir.AluOpType.add,
                )
                if prev_recip is not None:
                    tile.add_dep_helper(
                        red_i.ins, prev_recip.ins, sync=False,
                        reason="interleave reciprocal between reduces",
                    )
                with tc.high_priority():
                    # rnorm = sqrt(1/sumsq), emitted as fp16 matmul weights
                    rsum = small.tile([128, GB * CHUNKS], f32, tag="rsum")
                    prev_recip = nc.vector.reciprocal(out=rsum[:], in_=sumsq[:])
                    rnorm16 = small.tile([128, GB * CHUNKS], f16, tag="rnorm16")
                    sqrt_hist.append(
                        nc.scalar.activation(
                            out=rnorm16[:],
                            in_=rsum[:],
                            func=mybir.ActivationFunctionType.Sqrt,
                        )
                    )

                # s[b] = sum_q rnorm[q] * x[q, :], accumulated over the 4 chunks
                base = PS_BASES[g]
                for bb in range(GB):
                    out_slice = s_ps[0:1, base + bb * D : base + (bb + 1) * D]
                    for c in range(CHUNKS):
                        j = bb * CHUNKS + c
                        nc.tensor.matmul(
                            out_slice,
                            rnorm16[:, j : j + 1],
                            x_t[:, bb, c, :],
                            start=(c == 0),
                            stop=(c == CHUNKS - 1),
                        )

                # copy the PREVIOUS group's s vectors PSUM -> SBUF and ship
                # them to DRAM; the final -(1/Q^2)*||s||^2 runs on host.
                # Emitting the copy one group late puts it after this group's
                # reduce/recip/sqrt in each engine's instruction order, so
                # copies fill gaps instead of blocking the rnorm chain.
                # Alternate the copy engine to balance ACT vs DVE load.
                if g > 0:
                    _emit_copy_out(nc, fin, s_ps, s_d, g - 1)
                b0 += GB
            _emit_copy_out(nc, fin, s_ps, s_d, len(GROUP_SIZES) - 1)
    return nc


def _emit_copy_out(nc, fin, s_ps, s_d, g):
    import concourse.mybir as mybir

    f32 = mybir.dt.float32
    GB = GROUP_SIZES[g]
    b0 = sum(GROUP_SIZES[:g])
    base = PS_BASES[g]
    s_sb = fin.tile([1, GB * D], f32, tag="s_sb")
    ps_slice = s_ps[0:1, base : base + GB * D]
    if g % 2 == 0:
        nc.scalar.copy(s_sb[:], ps_slice)
    else:
        nc.vector.tensor_copy(s_sb[:], ps_slice)
    nc.sync.dma_start(
        out=s_d[b0 : b0 + GB].rearrange("b d -> (b d)").rearrange(
            "(a n) -> a n", a=1
        ),
        in_=s_sb[:],
    )


def _make_nc():
    import concourse.bacc as bacc

    nc = bacc.Bacc(trn_type="TRN2")
    _build(nc)
    # Bacc.finalize runs the legalization passes (wait splitting, matmul
    # wait->ldweights motion) that the TRN2 1-wait-per-instruction HW
    # constraint requires.
    nc.finalize()
    return nc


def _finish(s):
    # s: [BL, D] per-core matmul output; out[b] = -(1/Q^2) * sum_d s[b,d]^2
    s = s.astype(np.float32)
    return -(s * s).sum(axis=-1) / np.float32(Q * Q)


def _run(x, trace=False):
    from concourse.bass_utils import run_bass_kernel_spmd

    in_maps = [
        {"x": np.ascontiguousarray(x[i * BL : (i + 1) * BL])} for i in range(N_CORES)
    ]
    nc = _make_nc()
    res = run_bass_kernel_spmd(
        nc, in_maps, core_ids=list(range(N_CORES)), trace=trace
    )
    out = np.concatenate([_finish(r["s_out"]) for r in res.results], axis=0)
    return out.astype(np.float32), res


def kernel(x: np.ndarray) -> np.ndarray:
    out, _ = _run(np.asarray(x, dtype=np.float32))
    return out


# revision 20
# speedup vs baseline: 1.0608x; 1.0608x over previous
"""DisagreementRegularizer Trainium2 kernel.

reference math:
    xn = x / max(||x||_2 along d, eps)
    sim[b,q,p] = xn[b,q,:] . xn[b,p,:]
    out[b] = -mean_{q,p} sim  =  -(1/Q^2) * || sum_q xn[b,q,:] ||^2

Per batch b (on device):
    sumsq[q] = sum_d x[q,d]^2            (ACT Square big-call, DVE segmented reduce)
    rnorm[q] = sqrt(1/sumsq[q])          (DVE reciprocal + ACT Sqrt -> fp16)
    s[d]     = sum_q rnorm[q]*x[q,d]     (PE matmul, rnorm as stationary weights)
Host: out[b] = -(1/Q^2) * sum_d s[b,d]^2   (tiny: 16x256 per core)

All activations used (square, sqrt, copy) live in the single table set
sqrt_and_others; a dummy Sqrt is emitted first so that set is chosen at
the first activation -> exactly one ACT_TABLE_LOAD. The rnorm chain runs
under tc.high_priority() so the tiny ops that unblock the matmuls are
not scheduled behind later groups' big squares. Each group gets a
2KB-bank-aligned PSUM region (Tile's bank tracker is partition-blind,
so bank sharing between groups creates false serialization).

x is cast fp32->fp16 during the DMA load (SWDGE cast) so the matmuls run
single-pass at 1 cycle/row instead of fp32's two half-rate passes.

Sharding: pure data parallel, batch dim 128 -> 16 per core across 8 cores.
"""

import numpy as np

B, Q, D = 128, 512, 256
N_CORES = 8
BL = B // N_CORES  # 16 batches per core
CHUNKS = 4  # Q = 512 = 128 partitions x 4 chunks
# 2-batch groups pace the pipeline finely; 1-batch tail groups shorten the
# serial chain after the last DMA lands
GROUP_SIZES = [2, 2, 2, 2, 2, 2, 2, 1, 1]
# 512-col (2KB PSUM bank) aligned column base for each group's s vectors;
# only g7/g8 share a bank (adjacent in time anyway)
PS_BASES = [0, 512, 1024, 1536, 2048, 2560, 3072, 3584, 3840]
EPS = 1e-12


def _build(nc):
    import concourse.mybir as mybir
    import concourse.tile as tile

    f32 = mybir.dt.float32
    f16 = mybir.dt.float16

    x_d = nc.dram_tensor("x", [BL, Q, D], f32, kind="ExternalInput").ap()
    s_d = nc.dram_tensor("s_out", [BL, D], f32, kind="ExternalOutput").ap()

    with tile.TileContext(nc) as tc:
        with (
            tc.tile_pool(name="xp", bufs=1) as xp,
            tc.tile_pool(name="sqp", bufs=4) as sqp,
            tc.tile_pool(name="small", bufs=2 * len(GROUP_SIZES)) as small,
            tc.tile_pool(name="fin", bufs=3) as fin,
            tc.tile_pool(name="ps", bufs=1, space="PSUM") as psp,
        ):
            s_ps = psp.tile([1, BL * D], f32)  # 16 * 256 = 4096 fp32 = all of PSUM

            # issue every group's load upfront so the DMA stream is dense
            # from the first possible cycle; cast fp32->fp16 in the DMA.
            # partition p holds rows 4p..4p+3 of each batch.
            x_tiles = []
            b0 = 0
            for g, GB in enumerate(GROUP_SIZES):
                x_t = xp.tile([128, GB, CHUNKS, D], f16, tag=f"x_t{g}")
                src = x_d[b0 : b0 + GB].rearrange("b (p c) d -> p b c d", p=128)
                nc.gpsimd.dma_start(out=x_t[:], in_=src)
                x_tiles.append(x_t)
                b0 += GB

            # dummy Sqrt: pins the activation table set to sqrt_and_others
            # (which also contains square and copy)
            dummy = small.tile([1, 1], f32, tag="dummy")
            nc.vector.memset(dummy[:], 1.0)
            dummy2 = small.tile([1, 1], f32, tag="dummy2")
            nc.scalar.activation(
                out=dummy2[:], in_=dummy[:], func=mybir.ActivationFunctionType.Sqrt
            )

            b0 = 0
            prev_recip = None
            sqrt_hist = []
            for g, GB in enumerate(GROUP_SIZES):
                x_t = x_tiles[g]
                # square the whole group tile in one ACT call
                sq = sqp.tile([128, GB * CHUNKS * D], f16, tag="sq")
                sq_i = nc.scalar.activation(
                    out=sq[:],
                    in_=x_t[:].rearrange("p b c d -> p (b c d)"),
                    func=mybir.ActivationFunctionType.Square,
                )
                if len(sqrt_hist) >= 2:
                    # keep ACT interleaved with 2 groups of slack: the tiny
                    # sqrt of group g-2 must not get scheduled behind this
                    # (and every later) square, but forcing g-1's would
                    # serialize ACT against the DVE reduce chain
                    tile.add_dep_helper(
                        sq_i.ins, sqrt_hist[-2].ins, sync=False,
                        reason="interleave rnorm sqrt between squares",
                    )
                # segmented reduce -> sumsq per row (GB*4 segments of 256)
                sumsq = small.tile([128, GB * CHUNKS], f32, tag="sumsq")
                red_i = nc.vector.tensor_reduce(
                    out=sumsq[:],
                    in_=sq[:].rearrange("p (s d) -> p s d", d=D),
                    axis=mybir.AxisListType.X,
                    op=mybir.AluOpType.add,
                )
                if prev_recip is not None:
                    tile.add_dep_helper(
                        red_i.ins, prev_recip.ins, sync=False,
                        reason="interleave reciprocal between reduces",
                    )
                with tc.high_priority():
                    # rnorm = sqrt(1/sumsq), emitted as fp16 matmul weights
                    rsum = small.tile([128, GB * CHUNKS], f32, tag="rsum")
                    prev_recip = nc.vector.reciprocal(out=rsum[:], in_=sumsq[:])
                    rnorm16 = small.tile([128, GB * CHUNKS], f16, tag="rnorm16")
                    sqrt_hist.append(
                        nc.scalar.activation(
                            out=rnorm16[:],
                            in_=rsum[:],
                            func=mybir.ActivationFunctionType.Sqrt,
                        )
                    )

                # s[b] = sum_q rnorm[q] * x[q, :], accumulated over the 4 chunks
                base = PS_BASES[g]
                for bb in range(GB):
                    out_slice = s_ps[0:1, base + bb * D : base + (bb + 1) * D]
                    for c in range(CHUNKS):
                        j = bb * CHUNKS + c
                        nc.tensor.matmul(
                            out_slice,
                            rnorm16[:, j : j + 1],
                            x_t[:, bb, c, :],
                            start=(c == 0),
                            stop=(c == CHUNKS - 1),
                        )

                # copy the PREVIOUS group's s vectors PSUM -> SBUF and ship
                # them to DRAM; the final -(1/Q^2)*||s||^2 runs on host.
                # Emitting the copy one group late puts it after this group's
                # reduce/recip/sqrt in each engine's instruction order, so
                # copies fill gaps instead of blocking the rnorm chain.
                # Alternate the copy engine to balance ACT vs DVE load.
                if g > 0:
                    _emit_copy_out(nc, fin, s_ps, s_d, g - 1)
                b0 += GB
            _emit_copy_out(nc, fin, s_ps, s_d, len(GROUP_SIZES) - 1)
    return nc


def _emit_copy_out(nc, fin, s_ps, s_d, g):
    import concourse.mybir as mybir

    f32 = mybir.dt.float32
    GB = GROUP_SIZES[g]
    b0 = sum(GROUP_SIZES[:g])
    base = PS_BASES[g]
    s_sb = fin.tile([1, GB * D], f32, tag="s_sb")
    ps_slice = s_ps[0:1, base : base + GB * D]
    if g % 2 == 0:
        nc.scalar.copy(s_sb[:], ps_slice)
    else:
        nc.vector.tensor_copy(s_sb[:], ps_slice)
    nc.sync.dma_start(
        out=s_d[b0 : b0 + GB].rearrange("b d -> (b d)").rearrange(
            "(a n) -> a n", a=1
        ),
        in_=s_sb[:],
    )


def _make_nc():
    import concourse.bacc as bacc

    nc = bacc.Bacc(trn_type="TRN2")
    _build(nc)
    # Bacc.finalize runs the legalization passes (wait splitting, matmul
    # wait->ldweights motion) that the TRN2 1-wait-per-instruction HW
    # constraint requires.
    nc.finalize()
    return nc


def _finish(s):
    # s: [BL, D] per-core matmul output; out[b] = -(1/Q^2) * sum_d s[b,d]^2
    s = s.astype(np.float32)
    return -(s * s).sum(axis=-1) / np.float32(Q * Q)


def _run(x, trace=False):
    from concourse.bass_utils import run_bass_kernel_spmd

    in_maps = [
        {"x": np.ascontiguousarray(x[i * BL : (i + 1) * BL])} for i in range(N_CORES)
    ]
    nc = _make_nc()
    res = run_bass_kernel_spmd(
        nc, in_maps, core_ids=list(range(N_CORES)), trace=trace
    )
    out = np.concatenate([_finish(r["s_out"]) for r in res.results], axis=0)
    return out.astype(np.float32), res


def kernel(x: np.ndarray) -> np.ndarray:
    out, _ = _run(np.asarray(x, dtype=np.float32))
    return out
